# revision 52
# baseline (speedup 1.0000x reference)
"""DeltaGRU Trainium2 kernel: 2-layer delta-GRU (H=512) over T=1024, B=64.

Strategy: data-parallel over batch across 8 NeuronCores (8 samples/core),
weights replicated. Per core one Bass kernel runs the whole recurrence:

- State kept H-major (H on partitions, batch on free dim) so gate math runs on
  [128, *] tiles.
- Matmuls: mac.T = W @ delta with weight tiles stationary as lhsT. Weights and
  per-tick deltas are split hi/lo into bf16 pairs. The delta hi/lo halves are
  PACKED side-by-side in one [128, 16] moving operand, so each logical fp32
  matmul is 2 self-loading bf16 matmuls (Whi@[dhi|dlo], Wlo@[dhi|dlo]) instead
  of 3 narrow ones - the recurrent loop is PE-instruction-issue-bound
  (~24 ns/instruction), so fewer/wider instructions are faster. PSUM keeps the
  hi and lo partial sums in adjacent 8-column halves; the gate stage adds the
  halves when reading PSUM (all 4 product terms kept: more accurate than the
  old 3-term scheme).
- The GRU delta-memory (dm, dm_nh) lives permanently in PSUM; each tick's
  matmuls accumulate into it (start=False); one bank claim at init.
- Input feature expansion (i, q, amp, amp^3, q/amp, i/amp) computed on device
  in a pre-pass; the recurrent loop performs zero DMA.
- One For_i dynamic loop over the 1024 timesteps.

kernel(**inputs) takes the full unsharded inputs, returns [64, 1024, 2] f32.
"""
import numpy as np
import ml_dtypes
import concourse.bass as bass
import concourse.tile as tile
import concourse.mybir as mybir
from concourse import bacc
from concourse.bass_utils import run_bass_kernel_spmd

dt = mybir.dt
Alu = mybir.AluOpType
Act = mybir.ActivationFunctionType

H = 512
KT = H // 128
TH_X = 0.1
TH_H = 0.05
B, T, OUT = 64, 1024, 2
NCORES = 8
BC = B // NCORES


def _bias_layout(bc):
    bo = {}
    cur = 0
    for l in range(2):
        bo[("rz", l)] = cur; cur += 2 * H
        bo[("n", l)] = cur; cur += H
        bo[("nh", l)] = cur; cur += H
    bo["fc"] = cur; cur += 2
    bo["ones"] = cur; cur += bc
    return bo, cur


def _weight_mats(inp):
    return (
        np.ascontiguousarray(np.asarray(inp["W_hh_l0"], np.float32).T),
        np.ascontiguousarray(np.asarray(inp["W_ih_l0"], np.float32).T),
        np.ascontiguousarray(np.asarray(inp["W_ih_l1"], np.float32).T),
        np.ascontiguousarray(np.asarray(inp["W_hh_l1"], np.float32).T),
    )


def _pack_weights_fp(inp, bc):
    """fp32 blob: FC tiles + bias/ones row (hidden weights live in the bf16 blob)."""
    cols = []
    Wfc = np.ascontiguousarray(np.asarray(inp["W_fc"], np.float32).T)
    fc = np.zeros((128, 4 * 2), np.float32)
    for k in range(KT):
        fc[:, k * 2:(k + 1) * 2] = Wfc[k * 128:(k + 1) * 128, :]
    cols.append(fc)

    bo, blen = _bias_layout(bc)
    brow = np.zeros((blen,), np.float32)
    for l in range(2):
        b_ih = np.asarray(inp[f"b_ih_l{l}"], np.float32)
        b_hh = np.asarray(inp[f"b_hh_l{l}"], np.float32)
        brow[bo[("rz", l)]:bo[("rz", l)] + 2 * H] = np.concatenate(
            [b_ih[:H] + b_hh[:H], b_ih[H:2 * H] + b_hh[H:2 * H]])
        brow[bo[("n", l)]:bo[("n", l)] + H] = b_ih[2 * H:]
        brow[bo[("nh", l)]:bo[("nh", l)] + H] = b_hh[2 * H:]
    brow[bo["fc"]:bo["fc"] + 2] = np.asarray(inp["b_fc"], np.float32)
    brow[bo["ones"]:bo["ones"] + bc] = 1.0
    bt = np.zeros((128, blen), np.float32)
    bt[0, :] = brow
    cols.append(bt)
    return np.ascontiguousarray(np.concatenate(cols, axis=1))


def _pack_weights_bf(inp):
    """bf16 hi/lo tile blob; order mirrors _emit's wtb bookkeeping."""
    cols = []

    def tile_pair(mat_T, k0, kn, m0):
        t = np.zeros((128, 128), np.float32)
        t[:kn, :] = mat_T[k0:k0 + kn, m0:m0 + 128]
        hi = t.astype(ml_dtypes.bfloat16)
        lo = (t - hi.astype(np.float32)).astype(ml_dtypes.bfloat16)
        return hi, lo

    Whh0, Wih0, Wih1, Whh1 = _weight_mats(inp)
    for row0 in (0, H, 2 * H):
        for m in range(4):
            hi, lo = tile_pair(Wih0, 0, 6, row0 + m * 128)
            cols.append(hi); cols.append(lo)
    return np.ascontiguousarray(np.concatenate(cols, axis=1))


def _pack_weights_f16(inp):
    """W_hh_l0/W_hh_l1/W_ih_l1 as single fp16 tiles (their moving operands
    dhl/dx1l stay hi/lo-packed fp16, which the accuracy NEEDS - see memory:
    single-single fp16 breaks the threshold dynamics. Weight-side fp16 is
    free: the chaos floor dominates its rounding)."""
    Whh0, _, Wih1, Whh1 = _weight_mats(inp)
    cols = []
    for W in (Whh0, Whh1):
        for row0 in (0, H, 2 * H):
            for m in range(4):
                for k in range(KT):
                    cols.append(np.ascontiguousarray(
                        W[k * 128:(k + 1) * 128, row0 + m * 128:row0 + m * 128 + 128]
                    ).astype(np.float16))
    for row0 in (0, H, 2 * H):
        for m in range(4):
            for k in range(KT):
                t = np.zeros((128, 128), np.float32)
                t[:, :] = Wih1[k * 128:(k + 1) * 128, row0 + m * 128:row0 + m * 128 + 128]
                cols.append(t.astype(np.float16))
    return np.ascontiguousarray(np.concatenate(cols, axis=1))


def _wcols_fp(bc):
    _, blen = _bias_layout(bc)
    return 8 + blen


def _wbcols():
    return (3 * 4) * 2 * 128


def _wfcols():
    return (2 * 3 * 4 * KT + 3 * 4 * KT) * 128


def _build_kernel(T_, bc):
    nc = bacc.Bacc("TRN2", target_bir_lowering=False)
    x_d = nc.dram_tensor("xin", [2, T_ * bc], dt.float32, kind="ExternalInput")
    w_d = nc.dram_tensor("wblob", [128, _wcols_fp(bc)], dt.float32, kind="ExternalInput")
    wb_d = nc.dram_tensor("wbblob", [128, _wbcols()], dt.bfloat16, kind="ExternalInput")
    wf_d = nc.dram_tensor("wfblob", [128, _wfcols()], dt.float16, kind="ExternalInput")
    o_d = nc.dram_tensor("out", [2, T_ * bc], dt.float32, kind="ExternalOutput")
    with tile.TileContext(nc) as tc:
        _emit(nc, tc, x_d, w_d, wb_d, wf_d, o_d, T_, bc)
    nc.finalize()
    return nc


def _emit(nc, tc, x_d, w_d, wb_d, wf_d, o_d, T, bc):
    import contextlib
    ctx = contextlib.ExitStack()
    sb = ctx.enter_context(tc.tile_pool(name="sb", bufs=1))
    ps = ctx.enter_context(tc.tile_pool(name="ps", bufs=1, space="PSUM"))
    bc2 = 2 * bc  # packed hi|lo moving width

    w_s = sb.tile([128, _wcols_fp(bc)], dt.float32, tag="wblob")
    nc.gpsimd.dma_start(w_s[:], w_d[:, :])
    wb_s = sb.tile([128, _wbcols()], dt.bfloat16, tag="wbblob")
    nc.gpsimd.dma_start(wb_s[:], wb_d[:, :])
    wf_s = sb.tile([128, _wfcols()], dt.float16, tag="wfblob")
    nc.gpsimd.dma_start(wf_s[:], wf_d[:, :])
    feat = sb.tile([8, T * bc], dt.float32, tag="feat")

    fc_tiles = w_s[:, 0:8]
    bias_off = 8
    bo, _ = _bias_layout(bc)

    def bias_ap(start, ln):
        return w_s[0:1, bias_off + start: bias_off + start + ln]

    ones = bias_ap(bo["ones"], bc)

    wtb = {}
    boff = [0]

    def next_btile():
        ap = wb_s[:, boff[0]:boff[0] + 128]
        boff[0] += 128
        return ap

    foff = [0]

    def next_ftile():
        ap = wf_s[:, foff[0]:foff[0] + 128]
        foff[0] += 128
        return ap

    for l in range(2):
        for g in ("r", "z", "nh"):
            for m in range(4):
                for k in range(KT):
                    wtb[("hh", l, g, m, k)] = next_ftile()
    for g in ("r", "z", "n"):
        for m in range(4):
            wtb[("ih", 0, g, m, 0)] = (next_btile(), next_btile())
    for g in ("r", "z", "n"):
        for m in range(4):
            for k in range(KT):
                wtb[("ih", 1, g, m, k)] = next_ftile()

    h = sb.tile([128, 2 * KT * bc], dt.float32, tag="h")
    hp = sb.tile([128, 2 * KT * bc], dt.float32, tag="hp")
    dh = sb.tile([128, 2 * KT * bc], dt.float32, tag="dh")
    sc = sb.tile([128, 2 * KT * bc], dt.float32, tag="sc")
    xp1 = sb.tile([128, KT * bc], dt.float32, tag="xp1")
    dx1 = sb.tile([128, KT * bc], dt.float32, tag="dx1")
    sc1 = sb.tile([128, KT * bc], dt.float32, tag="sc1")
    xp0 = sb.tile([8, bc], dt.float32, tag="xp0")
    dx0 = sb.tile([8, bc], dt.float32, tag="dx0")
    sc0 = sb.tile([8, bc], dt.float32, tag="sc0")
    rz_s = sb.tile([128, 8 * bc], dt.float32, tag="rzs")
    rzsum = sb.tile([128, 8 * bc], dt.float32, tag="rzsum")
    nsum = sb.tile([128, 2 * KT * bc], dt.float32, tag="nsum")
    gsum_rz = sb.tile([128, 8 * bc2], dt.float32, tag="gsumrz")
    gsum_n = sb.tile([128, 8 * bc2], dt.float32, tag="gsumn")
    a_s = sb.tile([128, KT * bc], dt.float32, tag="as")
    b_s = sb.tile([128, KT * bc], dt.float32, tag="bs")
    n_s = sb.tile([128, KT * bc], dt.float32, tag="ns")
    u_s = sb.tile([128, KT * bc], dt.float32, tag="us")
    outring = sb.tile([2, T * bc], dt.float32, tag="outring")
    # packed bf16 moving operands: per (layer,k) group 16 cols = [hi(8)|lo(8)]
    dhl = sb.tile([128, 2 * KT * bc2], dt.float16, tag="dhl")
    dx1l = sb.tile([128, KT * bc2], dt.float16, tag="dx1l")
    dx0l = sb.tile([8, bc2], dt.bfloat16, tag="dx0l")

    def hsl(l):
        return h[:, l * KT * bc:(l + 1) * KT * bc]

    def hks(l, k):
        return h[:, (l * KT + k) * bc:(l * KT + k + 1) * bc]

    ps_rz = []
    for l in range(2):
        t = ps.tile([128, 8 * bc2], dt.float32, tag=f"psrz{l}", name=f"psrz{l}")
        ps_rz.append(t)
    ps_n = []
    for l in range(2):
        t = ps.tile([128, 8 * bc2], dt.float32, tag=f"psn{l}", name=f"psn{l}")
        ps_n.append(t)
    ps_fc = ps.tile([2, bc], dt.float32, tag="psfc")

    def rz_ps(l, g, m):
        return ps_rz[l][:, (g * 4 + m) * bc2:(g * 4 + m + 1) * bc2]

    def n_ps(l, g, m):
        return ps_n[l][:, (g * 4 + m) * bc2:(g * 4 + m + 1) * bc2]

    def rz_ps_hi(l, g, m):
        return ps_rz[l][:, (g * 4 + m) * bc2:(g * 4 + m) * bc2 + bc]

    def n_ps_hi(l, g, m):
        return ps_n[l][:, (g * 4 + m) * bc2:(g * 4 + m) * bc2 + bc]

    # feature expansion on [128, N/128] tiles, scattered into feat rows via DMA
    N = T * bc
    FCW = N // 128
    xi = sb.tile([128, FCW], dt.float32, tag="xi")
    xq = sb.tile([128, FCW], dt.float32, tag="xq")
    fs = sb.tile([128, FCW], dt.float32, tag="fs")
    fv = sb.tile([128, FCW], dt.float32, tag="fv")
    famp = sb.tile([128, FCW], dt.float32, tag="famp")
    famp3 = sb.tile([128, FCW], dt.float32, tag="famp3")
    fqn = sb.tile([128, FCW], dt.float32, tag="fqn")
    fin = sb.tile([128, FCW], dt.float32, tag="fin")
    nc.gpsimd.dma_start(xi[:], x_d[0:1, :].rearrange("o (p c) -> (o p) c", p=128))
    nc.gpsimd.dma_start(xq[:], x_d[1:2, :].rearrange("o (p c) -> (o p) c", p=128))
    nc.vector.tensor_tensor(fs[:], xi[:], xi[:], Alu.mult)
    nc.vector.tensor_tensor(fv[:], xq[:], xq[:], Alu.mult)
    nc.vector.tensor_tensor(fs[:], fs[:], fv[:], Alu.add)
    nc.scalar.activation(fv[:], fs[:], Act.Abs_reciprocal_sqrt)
    nc.vector.tensor_tensor(famp[:], fs[:], fv[:], Alu.mult)
    nc.vector.tensor_tensor(famp3[:], fs[:], famp[:], Alu.mult)
    nc.vector.tensor_tensor(fqn[:], xq[:], fv[:], Alu.mult)
    nc.vector.tensor_tensor(fin[:], xi[:], fv[:], Alu.mult)
    nc.vector.memset(feat[:], 0.0)
    for f, src_t in enumerate((xi, xq, famp, famp3, fqn, fin)):
        nc.gpsimd.dma_start(feat[f:f + 1, :], src_t[:])

    nc.vector.memset(h[:], 0.0)
    nc.vector.memset(hp[:], 0.0)
    nc.vector.memset(xp1[:], 0.0)
    nc.vector.memset(xp0[:], 0.0)
    for l in range(2):
        for m in range(4):
            nc.tensor.matmul(rz_ps_hi(l, 0, m), bias_ap(bo[("rz", l)] + m * 128, 128), ones, start=(m == 0), stop=False)
            nc.tensor.matmul(rz_ps_hi(l, 1, m), bias_ap(bo[("rz", l)] + H + m * 128, 128), ones, start=False, stop=False)
            nc.tensor.matmul(n_ps_hi(l, 0, m), bias_ap(bo[("n", l)] + m * 128, 128), ones, start=(m == 0), stop=False)
            nc.tensor.matmul(n_ps_hi(l, 1, m), bias_ap(bo[("nh", l)] + m * 128, 128), ones, start=False, stop=False)

    ones_t = sb.tile([128, KT * bc], dt.float32, tag="onest")
    nc.vector.memset(ones_t[:], 1.0)

    def delta_block(d_out, scr, cur, prev, th):
        nc.vector.tensor_tensor(scr, cur, prev, Alu.subtract)
        nc.vector.tensor_tensor(d_out, scr, scr, Alu.mult)
        nc.vector.scalar_tensor_tensor(d_out, d_out, th * th, scr, Alu.is_ge, Alu.mult)
        nc.vector.tensor_tensor(prev, prev, d_out, Alu.add)

    def pack_hi_lo(src_f32, dst, ngroups):
        """src [p, ngroups*bc] f32 -> dst [p, ngroups*bc2] bf16 as [hi|lo] per group."""
        dv = dst.rearrange("p (g c) -> p g c", c=bc2)
        hi = dv[:, :, 0:bc]
        lo = dv[:, :, bc:bc2]
        nc.vector.tensor_copy(hi, src_f32)
        nc.vector.tensor_tensor(lo, src_f32, hi, Alu.subtract)

    def mm2(out_ps, key, rhs16, l):
        whi, wlo = wtb[key]
        if l == 0 and key[0] == "ih":
            whi = whi[0:6, :]; wlo = wlo[0:6, :]
        nc.tensor.matmul(out_ps, whi, rhs16, start=False, stop=False)
        return nc.tensor.matmul(out_ps, wlo, rhs16, start=False, stop=False)

    def layer_mms_hh(l, rhs_hh):
        last = None
        for m in range(4):
            for g, gi in (("r", 0), ("z", 1)):
                for k in range(KT):
                    nc.tensor.matmul(rz_ps(l, gi, m), wtb[("hh", l, g, m, k)], rhs_hh(k), start=False, stop=False)
            for k in range(KT):
                last = nc.tensor.matmul(n_ps(l, 1, m), wtb[("hh", l, "nh", m, k)], rhs_hh(k), start=False, stop=False)
        return last

    def layer_mms_ih(l, rhs_in, kin):
        for m in range(4):
            for g, gi in (("r", 0), ("z", 1)):
                for k in range(kin):
                    if l == 1:
                        nc.tensor.matmul(rz_ps(l, gi, m), wtb[("ih", 1, g, m, k)], rhs_in(k), start=False, stop=False)
                    else:
                        mm2(rz_ps(l, gi, m), ("ih", l, g, m, k), rhs_in(k), l)
            for k in range(kin):
                if l == 1:
                    nc.tensor.matmul(n_ps(l, 0, m), wtb[("ih", 1, "n", m, k)], rhs_in(k), start=False, stop=False)
                else:
                    mm2(n_ps(l, 0, m), ("ih", l, "n", m, k), rhs_in(k), l)

    def gates(l):
        # collapse the hi|lo PSUM column halves (DVE may read only one PSUM
        # operand per op: copy to SBUF first), then activations.
        # tanh(x) is computed as 2*sigmoid(2x)-1 so sigmoid's ACT table serves
        # every activation - no per-tick ACT_TABLE_LOAD on the critical path.
        nc.vector.tensor_copy(gsum_rz[:], ps_rz[l][:])
        rzv = gsum_rz.rearrange("p (g c) -> p g c", c=bc2)
        nc.vector.tensor_tensor(rzsum[:], rzv[:, :, 0:bc], rzv[:, :, bc:bc2], Alu.add)
        nc.vector.tensor_copy(gsum_n[:], ps_n[l][:])
        nv = gsum_n.rearrange("p (g c) -> p g c", c=bc2)
        nc.vector.tensor_tensor(nsum[:], nv[:, :, 0:bc], nv[:, :, bc:bc2], Alu.add)
        nc.scalar.activation(rz_s[:], rzsum[:], Act.Sigmoid)
        rpart = rz_s[:, 0:KT * bc]
        zpart = rz_s[:, KT * bc:2 * KT * bc]
        # b = 2*(dm_n + r*dm_nh);  s = sigmoid(b);  n = 2s - 1
        nc.vector.scalar_tensor_tensor(a_s[:], rpart, 2.0, nsum[:, KT * bc:2 * KT * bc], Alu.mult, Alu.mult)
        nc.vector.scalar_tensor_tensor(b_s[:], nsum[:, 0:KT * bc], 2.0, a_s[:], Alu.mult, Alu.add)
        nc.scalar.activation(n_s[:], b_s[:], Act.Sigmoid)
        nc.vector.scalar_tensor_tensor(n_s[:], n_s[:], 2.0, ones_t[:], Alu.mult, Alu.subtract)
        nc.vector.tensor_tensor(u_s[:], hsl(l), n_s[:], Alu.subtract)
        nc.vector.tensor_tensor(u_s[:], zpart, u_s[:], Alu.mult)
        nc.vector.tensor_tensor(hsl(l), n_s[:], u_s[:], Alu.add)

    def dh_l(l):
        return dh[:, l * KT * bc:(l + 1) * KT * bc]

    def fc_block(ivpj, after=None):
        bias_mm = nc.tensor.matmul(ps_fc[:], bias_ap(bo["fc"], 2), ones, start=True, stop=False)
        if after is not None:
            # ordering-only edge: keep the FC matmuls (which wait on the full
            # gates(1) chain) behind this tick's hh-l1 stream in the PE FIFO -
            # the scheduler's cost model under-estimates the DVE chain and
            # otherwise hoists FC right after l0, stalling the PE ~3us/tick.
            tile.add_dep_helper(bias_mm.ins, after.ins, sync=False,
                                reason="fc after hh-l1 stream")
        for k in range(KT):
            nc.tensor.matmul(ps_fc[:], fc_tiles[:, k * 2:(k + 1) * 2], hks(1, k), start=False, stop=(k == KT - 1))
        nc.vector.tensor_copy(outring[:, bass.ds(ivpj * bc, bc)], ps_fc[:])

    def one_tick(ivpj, prev_fc):
        """ivpj: ScalarValue index of this tick. Emission order is the
        intended execution order: both layers' dh deltas first so the l1 hh
        matmul stream can cover the gates(0)/dx1 vector chain. The previous
        tick's FC matmuls are emitted after this tick's layer-0 matmuls so
        they don't stall the PE FIFO on the previous gates(1) chain (h1 is
        not overwritten until this tick's gates(1), far later in the order)."""
        xt8 = feat[0:8, bass.ds(ivpj * bc, bc)]
        # layer-0 dh delta + pack (PE blocked only on this)
        nc.vector.tensor_tensor(sc[:, 0:KT * bc], h[:, 0:KT * bc], hp[:, 0:KT * bc], Alu.subtract)
        nc.vector.tensor_tensor(dh[:, 0:KT * bc], sc[:, 0:KT * bc], sc[:, 0:KT * bc], Alu.mult)
        nc.vector.scalar_tensor_tensor(dh[:, 0:KT * bc], dh[:, 0:KT * bc], TH_H * TH_H, sc[:, 0:KT * bc], Alu.is_ge, Alu.mult)
        nc.vector.tensor_tensor(hp[:, 0:KT * bc], hp[:, 0:KT * bc], dh[:, 0:KT * bc], Alu.add)
        pack_hi_lo(dh_l(0), dhl[:, 0:KT * bc2], KT)
        delta_block(dx0[:], sc0[:], xt8, xp0[:], TH_X)
        pack_hi_lo(dx0[:], dx0l[:], 1)
        # layer-1 dh delta + pack (independent of layer-0 gates; overlaps PE)
        nc.vector.tensor_tensor(sc[:, KT * bc:], h[:, KT * bc:], hp[:, KT * bc:], Alu.subtract)
        nc.vector.tensor_tensor(dh[:, KT * bc:], sc[:, KT * bc:], sc[:, KT * bc:], Alu.mult)
        nc.vector.scalar_tensor_tensor(dh[:, KT * bc:], dh[:, KT * bc:], TH_H * TH_H, sc[:, KT * bc:], Alu.is_ge, Alu.mult)
        nc.vector.tensor_tensor(hp[:, KT * bc:], hp[:, KT * bc:], dh[:, KT * bc:], Alu.add)
        pack_hi_lo(dh[:, KT * bc:], dhl[:, KT * bc2:], KT)

        layer_mms_hh(0, lambda k: dhl[:, k * bc2:(k + 1) * bc2])
        layer_mms_ih(0, lambda k: dx0l[0:6, :], 1)
        layer_mms_hh(1, lambda k: dhl[:, (KT + k) * bc2:(KT + k + 1) * bc2])
        if prev_fc is not None:
            # note: an explicit add_dep_helper(fc, last-hh1) edge was tried to
            # pin FC behind the hh-l1 stream; it measured ~1.1ms SLOWER
            # (16.64ms vs 15.55ms) - the scheduler's natural order wins.
            fc_block(prev_fc)
        gates(0)
        delta_block(dx1[:], sc1[:], hsl(0), xp1[:], TH_X)
        pack_hi_lo(dx1[:], dx1l[:], KT)
        layer_mms_ih(1, lambda k: dx1l[:, k * bc2:(k + 1) * bc2], KT)
        gates(1)
        return ivpj

    UNROLL = 8
    with tc.For_i(0, T, UNROLL, hint_engines=(mybir.EngineType.PE, mybir.EngineType.DVE)) as iv:
        pending_fc = None
        for j in range(UNROLL):
            pending_fc = one_tick(iv + j, pending_fc)
        fc_block(pending_fc)

    nc.gpsimd.dma_start(o_d[:, :], outring[:])


_NC_CACHE = {}
_LAST_RES = None


def kernel(**inputs) -> np.ndarray:
    x = np.asarray(inputs["x"], np.float32)            # [64, 1024, 2]
    wblob = _pack_weights_fp(inputs, BC)
    wbblob = _pack_weights_bf(inputs)
    wfblob = _pack_weights_f16(inputs)
    if ("k", T, BC) not in _NC_CACHE:
        _NC_CACHE[("k", T, BC)] = _build_kernel(T, BC)
    nc = _NC_CACHE[("k", T, BC)]

    in_maps = []
    for c in range(NCORES):
        xs = x[c * BC:(c + 1) * BC]                    # [bc, T, 2]
        xin = np.ascontiguousarray(xs.transpose(2, 1, 0).reshape(2, T * BC))
        in_maps.append({"xin": xin, "wblob": wblob, "wbblob": wbblob, "wfblob": wfblob})

    res = run_bass_kernel_spmd(nc, in_maps, core_ids=list(range(NCORES)))
    global _LAST_RES
    _LAST_RES = res
    outs = []
    for c in range(NCORES):
        o = res.results[c]["out"]                      # [2, T*bc]
        outs.append(np.ascontiguousarray(o.reshape(2, T, BC).transpose(2, 1, 0)))
    return np.concatenate(outs, axis=0).astype(np.float32)
